# revision 24
# baseline (speedup 1.0000x reference)
"""Trainium2 Bass kernel for batched multi-head self-attention.

Problem: x [8, 1500, 768], 12 heads x 64 dims, torch-Linear style projections.
Strategy: data-parallel over batch (1 element per NeuronCore, 8 cores).

Per-core design (host pre-transposes everything; device does no transposes):
  - xT [768, 1500] bf16: projections contract over d on the partition axis.
  - All four weight matrices live in SBUF (loaded once, OUTSIDE the rep
    loop); Q^T / K^T for the current head-pair stay SBUF-resident too, so
    the steady-state body issues only 4 DMA instructions per rep
    (1 x-load + 3 output stores). DMA instructions carry a large fixed
    execution cost on this runtime, so everything else moves via
    engine-to-engine SBUF/PSUM traffic.
  - reps (for the delta-timing method) run as a HARDWARE loop
    (tc.For_i), so the program size - and hence the NEFF load cost per
    dispatch - is independent of the rep count: the reps-delta then
    measures pure per-rep device execution.
  - scores computed TRANSPOSED: scoresT[k, q] = K_h^T.T @ Q_h^T, two
    matmuls (one per head of the pair; contraction is dh=64).
  - exp on ScalarE straight out of PSUM ([128,1024] two-bank spans), no
    max subtraction (scores ~ N(0,1): fp32-safe).
  - softmax denominators ride as a 65th all-ones column of V inside the
    ctx matmul (ctxT psum = 64 ctx rows + 1 sums row).
  - normalization: reciprocal of the sums row, partition-broadcast via a
    1-contraction matmul against an all-ones stationary (PSUM out), then
    multiplied in during the ctx psum eviction. No DRAM roundtrips.
  - output projection consumes ctx_normT [e, s] directly; the bv/bo
    contribution is a constant row (softmax rows sum to 1) added on host.

Matmul operands are bf16 (full PE rate at any moving size, ~0.4% rel
error per matmul, well inside the 2e-2 gate); all PSUM accumulation is
fp32.
"""

import numpy as np
from contextlib import ExitStack

import ml_dtypes

import concourse.bass as bass
import concourse.bacc as bacc
import concourse.tile as tile
from concourse import mybir
from concourse import bass_utils

F32 = mybir.dt.float32
F32R = mybir.dt.float32r
BF16 = mybir.dt.bfloat16
AF = mybir.ActivationFunctionType
OP = mybir.AluOpType

P = 128
D = 768
H = 12
DH = 64
NE = D // P          # 6 e-chunks (head pairs)
ND = D // P          # 6 d-chunks
SCALE = 0.125
S_FULL = 1500
QB = 512
EH = 384             # half of D for the V projection moving dim


def _chunks(total, size):
    out = []
    o = 0
    while o < total:
        out.append((o, min(size, total - o)))
        o += size
    return out


def _qblocks(S):
    """Uniform 512-wide q-blocks; the last block overlaps the previous
    one (start S-512) so no ragged-column paths exist for S >= 512. The
    overlap region is recomputed with identical values (benign WAW; Tile
    orders the writes)."""
    if S <= QB:
        return [(0, S)]
    starts = list(range(0, S - QB + 1, QB))
    if starts[-1] + QB < S:
        starts.append(S - QB)
    return [(s, QB) for s in starts]


def build_attention(tc, ctx, xT, wqT, wkT, wvT, woT, bqs, out, S, reps=1):
    """Emit the single-core attention program.

    xT:  [D, S] bf16 DRAM     (x^T for this batch element)
    wqT/wkT/wvT/woT: [D, D] bf16 DRAM  (W.T of the torch-Linear weights)
    bqs: [P, NE] f32 DRAM     (0.125*bq laid out [partition, e-chunk])
    out: [S, D] f32 DRAM      (missing the constant bv@Wo.T+bo row)
    """
    nc = tc.nc
    SC = _chunks(S, P)            # k-chunks, e.g. 11x128 + 92
    QBS = _qblocks(S)
    NSC = len(SC)

    const = ctx.enter_context(tc.tile_pool(name="const", bufs=1))
    qkv = ctx.enter_context(tc.tile_pool(name="qkv", bufs=1))
    gen_ps = ctx.enter_context(tc.tile_pool(name="gen_ps", bufs=2, space="PSUM"))
    sc_ps = ctx.enter_context(tc.tile_pool(name="sc_ps", bufs=2, space="PSUM"))
    ctx_ps = ctx.enter_context(tc.tile_pool(name="ctx_ps", bufs=1, space="PSUM"))
    e_pool = ctx.enter_context(tc.tile_pool(name="epool", bufs=3))
    sr_pool = ctx.enter_context(tc.tile_pool(name="srp", bufs=2))
    ctxn_pool = ctx.enter_context(tc.tile_pool(name="ctxn", bufs=3))
    rc_pool = ctx.enter_context(tc.tile_pool(name="rcp", bufs=3))
    out_sb_pool = ctx.enter_context(tc.tile_pool(name="outsb", bufs=3))
    kt_pool = ctx.enter_context(tc.tile_pool(name="ktp", bufs=2))
    x_pool = ctx.enter_context(tc.tile_pool(name="xp", bufs=1))

    # ---- persistent operands, loaded ONCE (outside the rep loop) ----
    bq_sb = const.tile([P, NE], F32)
    nc.sync.dma_start(out=bq_sb[:], in_=bqs)
    w_sbs = {}
    for key, wdram in (("q", wqT), ("k", wkT), ("v", wvT), ("o", woT)):
        w_sb = const.tile([P, ND, D], BF16, name=f"w_{key}")
        for dc in range(ND):
            nc.sync.dma_start(out=w_sb[:, dc, :],
                              in_=wdram[dc * P:(dc + 1) * P, :])
        w_sbs[key] = w_sb
    ones_sb = const.tile([1, DH], F32)
    nc.vector.memset(ones_sb[:], 1.0)

    V = qkv.tile([P, NSC, H * (DH + 1)], BF16)   # per-head 65th ones column
    # Fill all of V with 1.0 once: the projection evictions overwrite the
    # 64 data columns per head, leaving column DH as the all-ones column
    # that accumulates softmax denominators in the ctx matmul. bf16 memset
    # isn't a valid ISA op, so memset the f32-bitcast view with the bit
    # pattern of two packed bf16 1.0s (0x3F803F80).
    two_bf16_ones = float(np.frombuffer(
        np.uint32(0x3F803F80).tobytes(), dtype=np.float32)[0])
    nc.vector.memset(V[:, :, :].bitcast(F32), two_bf16_ones)

    # reps as a HARDWARE loop: program size (NEFF size / per-dispatch
    # load cost) is independent of the rep count, so the reps-delta
    # wall-clock measurement isolates pure per-rep device execution.
    def body():
        _emit_body(tc, nc, xT, out, S, SC, QBS, NSC,
                   kt_pool, V, bq_sb, w_sbs, ones_sb, gen_ps, sc_ps,
                   ctx_ps, e_pool, sr_pool, ctxn_pool, rc_pool,
                   out_sb_pool, x_pool)

    if reps == 1:
        body()
    else:
        with tc.For_i(0, reps, 1):
            body()


def _emit_body(tc, nc, xT, out, S, SC, QBS, NSC,
               kt_pool, V, bq_sb, w_sbs, ones_sb, gen_ps, sc_ps,
               ctx_ps, e_pool, sr_pool, ctxn_pool, rc_pool, out_sb_pool,
               x_pool):

    # x^T for this rep: 6 flat 2D DMAs (contiguous source rows lower to
    # single hardware descriptors; a fancy 3D pattern costs ~30x more).
    xT_sb = x_pool.tile([P, ND, S], BF16, tag="xt", name="xT_sb")
    for dc in range(ND):
        nc.sync.dma_start(out=xT_sb[:, dc, :],
                          in_=xT[dc * P:(dc + 1) * P, :])

    SPAN = 6              # k-chunks per exp batch (NSC=12 -> 2 batches)

    def scores_kc(kc, q0, qw, kt_t, qt_t, sraw):
        """QK^T for one k-chunk + PSUM->SBUF eviction (fp16 staging).
        Evictions alternate DVE / ScalarE so the ~1.6us blocked-semaphore
        wake latency is split between two engines instead of serializing
        on one, and the batched exp frees ScalarE of per-k-chunk waits."""
        (k0, kw) = SC[kc]
        sp = sc_ps.tile([P, 1024], F32, tag="sc", name="sp")
        for hi in range(2):
            nc.tensor.matmul(
                sp[:kw, hi * 512:hi * 512 + qw],
                kt_t[hi * DH:(hi + 1) * DH, k0:k0 + kw],
                qt_t[hi * DH:(hi + 1) * DH, q0:q0 + qw],
                start=True, stop=True)
        si = kc % SPAN
        if qw == 512:
            if kc % 2 == 0:
                nc.vector.tensor_copy(out=sraw[:kw, si, :], in_=sp[:kw, :])
            else:
                nc.scalar.copy(out=sraw[:kw, si, :], in_=sp[:kw, :])
        else:
            for hi in range(2):
                dst = sraw[:kw, si, hi * 512:hi * 512 + qw]
                src = sp[:kw, hi * 512:hi * 512 + qw]
                if kc % 2 == 0:
                    nc.vector.tensor_copy(out=dst, in_=src)
                else:
                    nc.scalar.copy(out=dst, in_=src)

    def exp_span(qw, sraw, sp0, nsc_span):
        """One giant exp over a whole staged span (up to 6 k-chunks) --
        tens of ACT instructions per rep instead of 216. Only staged
        (initialized) regions are read: full-row chunks go in one AP, a
        ragged-row tail chunk gets its own."""
        e_all = e_pool.tile([P, SPAN, 1024], BF16, tag="e", name="e_all")
        nf = sum(1 for si in range(nsc_span) if SC[sp0 + si][1] == P)
        col_ranges = ([(0, 1024)] if qw == 512
                      else [(0, qw), (512, qw)])
        for (c0, cw) in col_ranges:
            if nf:
                nc.scalar.activation(out=e_all[:, 0:nf, c0:c0 + cw],
                                     in_=sraw[:, 0:nf, c0:c0 + cw],
                                     func=AF.Exp)
            if nf < nsc_span:
                kw = SC[sp0 + nf][1]
                nc.scalar.activation(out=e_all[:kw, nf:nf + 1, c0:c0 + cw],
                                     in_=sraw[:kw, nf:nf + 1, c0:c0 + cw],
                                     func=AF.Exp)
        return e_all

    def ctx_kc(kc, qw, pr, cps, e_all):
        (k0, kw) = SC[kc]
        si = kc % SPAN
        for hi in range(2):
            h = 2 * pr + hi
            nc.tensor.matmul(
                cps[:, hi * 512:hi * 512 + qw],
                V[:kw, kc, h * (DH + 1):(h + 1) * (DH + 1)],
                e_all[:kw, si, hi * 512:hi * 512 + qw],
                start=(kc == 0), stop=(kc == NSC - 1))

    def phase2_end(qw, pr, cps, cn):
        for hi in range(2):
            # reciprocal of the denominators row, then partition-broadcast
            # it across the 64 ctx rows with a 1-contraction matmul. The
            # ctx rows evict PSUM->SBUF in parallel (a DVE op may read at
            # most one PSUM operand, so the multiply pairs SBUF x PSUM).
            co = hi * 512
            rc = rc_pool.tile([1, 512], F32R, tag="rc", name="rc")
            # f32r out is storage-identical to f32 here; the reciprocal
            # itself computes in fp32 internally.
            with nc.allow_low_precision(reason="f32r reciprocal, fp32 bits"):
                nc.vector.reciprocal(out=rc[:, 0:qw],
                                     in_=cps[DH:DH + 1, co:co + qw])
            craw = rc_pool.tile([DH, 512], F32, tag="craw", name="craw")
            nc.vector.tensor_copy(out=craw[:, 0:qw], in_=cps[0:DH, co:co + qw])
            rb_ps = gen_ps.tile([P, 512], F32, tag="mm", name="rb_ps")
            nc.tensor.matmul(rb_ps[:DH, 0:qw], ones_sb[:, :].bitcast(F32R),
                             rc[:, 0:qw], start=True, stop=True)
            nc.vector.tensor_tensor(
                out=cn[hi * DH:(hi + 1) * DH, pr, 0:qw],
                in0=craw[:, 0:qw], in1=rb_ps[:DH, 0:qw], op=OP.mult)

    def phase2_pair(q0, qw, pr, kt_t, qt_t, cn, interleave_v=False):
        """Per-(pair, q-block) attention, software-pipelined: all scores
        of a span are computed and staged before its exp; all ctx matmuls
        run after both exps are in flight, so every PE wait on ACT output
        is already satisfied when it issues."""
        cps = ctx_ps.tile([DH + 1, 1024], F32, tag="ctx", name="cps")
        e_spans = []
        for sp0 in range(0, NSC, SPAN):
            nsc_span = min(SPAN, NSC - sp0)
            sraw = sr_pool.tile([P, SPAN, 1024], mybir.dt.float16,
                                tag="sr", name="sraw")
            for kc in range(sp0, sp0 + nsc_span):
                if interleave_v:
                    emit_v_chunk(kc, *SC[kc])
                scores_kc(kc, q0, qw, kt_t, qt_t, sraw)
            e_spans.append(exp_span(qw, sraw, sp0, nsc_span))
        for kc in range(NSC):
            ctx_kc(kc, qw, pr, cps, e_spans[kc // SPAN])
        phase2_end(qw, pr, cps, cn)

    def phase3(q0, qw, cn):
        for (s0, sw) in _chunks(qw, P):
            ot = out_sb_pool.tile([P, D], F32, tag="ot", name="ot")
            for (o0, ow) in ((0, 512), (512, 256)):
                op_t = gen_ps.tile([P, 512], F32, tag="mm", name="op_t")
                for ec in range(NE):
                    nc.tensor.matmul(
                        op_t[:sw, :ow],
                        cn[:, ec, s0:s0 + sw],
                        w_sbs["o"][:, ec, o0:o0 + ow],
                        start=(ec == 0), stop=(ec == NE - 1))
                nc.vector.tensor_copy(out=ot[:sw, o0:o0 + ow],
                                      in_=op_t[:sw, :ow])
            # flat 2D store: contiguous destination rows -> cheap DMA
            nc.sync.dma_start(out=out[q0 + s0:q0 + s0 + sw, :],
                              in_=ot[:sw, :])

    def emit_kq(ec):
        kt_t = kt_pool.tile([P, S], BF16, tag="kt", name=f"kt{ec}")
        qt_t = kt_pool.tile([P, S], BF16, tag="qt", name=f"qt{ec}")
        for kind in ("q", "k"):
            w_sb = w_sbs[kind]
            for (q0, qw) in QBS:
                ps = gen_ps.tile([P, 512], F32, tag="mm", name="kq_ps")
                for dc in range(ND):
                    nc.tensor.matmul(
                        ps[:, :qw],
                        w_sb[:, dc, ec * P:(ec + 1) * P],
                        xT_sb[:, dc, q0:q0 + qw],
                        start=(dc == 0), stop=(dc == ND - 1))
                if kind == "q":
                    nc.vector.tensor_scalar(
                        out=qt_t[:, q0:q0 + qw], in0=ps[:, :qw],
                        scalar1=SCALE, scalar2=bq_sb[:, ec:ec + 1],
                        op0=OP.mult, op1=OP.add)
                else:
                    nc.vector.tensor_copy(out=kt_t[:, q0:q0 + qw],
                                          in_=ps[:, :qw])
        return kt_t, qt_t

    def emit_v_chunk(sc, s0, sw):
        for eh in range(D // EH):
            ps = gen_ps.tile([P, 512], F32, tag="mm", name="v_ps")
            for dc in range(ND):
                nc.tensor.matmul(
                    ps[:sw, :EH],
                    xT_sb[:, dc, s0:s0 + sw],
                    w_sbs["v"][:, dc, eh * EH:(eh + 1) * EH],
                    start=(dc == 0), stop=(dc == ND - 1))
            vh = V[:sw, sc, :].rearrange("p (h w) -> p h w", w=DH + 1)
            nc.vector.tensor_copy(
                out=vh[:, eh * (EH // DH):(eh + 1) * (EH // DH), 0:DH],
                in_=ps[:sw, :EH].rearrange("p (h w) -> p h w", w=DH))

    # pr-major emission (emission order IS program order under Tile):
    # each head-pair's K/Q projection is followed by that pair's
    # attention over ALL q-blocks. The V pass interleaves chunk-by-chunk
    # with the very first pair's scores. Each q-block's output
    # projection is emitted right after its last pair.
    cns = [ctxn_pool.tile([P, NE, 512], BF16, tag="cn", name=f"cn{_q}")
           for _q in range(len(QBS))]
    for pr in range(NE):
        kt_t, qt_t = emit_kq(pr)
        for qi, (q0, qw) in enumerate(QBS):
            phase2_pair(q0, qw, pr, kt_t, qt_t, cns[qi],
                        interleave_v=(pr == 0 and qi == 0))
            if pr == NE - 1:
                phase3(q0, qw, cns[qi])


def build_nc(S=S_FULL, reps=1):
    nc = bacc.Bacc("TRN2", target_bir_lowering=False, debug=False,
                   enable_asserts=False, num_devices=1)
    xT = nc.dram_tensor("xT", [D, S], BF16, kind="ExternalInput").ap()
    wqT = nc.dram_tensor("wqT", [D, D], BF16, kind="ExternalInput").ap()
    wkT = nc.dram_tensor("wkT", [D, D], BF16, kind="ExternalInput").ap()
    wvT = nc.dram_tensor("wvT", [D, D], BF16, kind="ExternalInput").ap()
    woT = nc.dram_tensor("woT", [D, D], BF16, kind="ExternalInput").ap()
    bqs = nc.dram_tensor("bqs", [P, NE], F32, kind="ExternalInput").ap()
    out = nc.dram_tensor("out", [S, D], F32, kind="ExternalOutput").ap()
    with tile.TileContext(nc) as tc:
        with ExitStack() as ctx:
            build_attention(tc, ctx, xT, wqT, wkT, wvT, woT, bqs, out, S, reps)
    nc.compile()
    return nc


_NC_CACHE = {}


def _get_nc(S=S_FULL, reps=1):
    if (S, reps) not in _NC_CACHE:
        _NC_CACHE[(S, reps)] = build_nc(S, reps)
    return _NC_CACHE[(S, reps)]


def prep_inputs(x, Wq, bq, Wk, Wv, bv, Wo, bo):
    x = np.asarray(x, dtype=np.float32)
    Wq = np.asarray(Wq, dtype=np.float32)
    Wk = np.asarray(Wk, dtype=np.float32)
    Wv = np.asarray(Wv, dtype=np.float32)
    Wo = np.asarray(Wo, dtype=np.float32)
    bq = np.asarray(bq, dtype=np.float32)
    bv = np.asarray(bv, dtype=np.float32)
    bo = np.asarray(bo, dtype=np.float32)
    bf = ml_dtypes.bfloat16
    xT = np.ascontiguousarray(x.transpose(0, 2, 1)).astype(bf)
    base = {
        "wqT": np.ascontiguousarray(Wq.T).astype(bf),
        "wkT": np.ascontiguousarray(Wk.T).astype(bf),
        "wvT": np.ascontiguousarray(Wv.T).astype(bf),
        "woT": np.ascontiguousarray(Wo.T).astype(bf),
        "bqs": np.ascontiguousarray((SCALE * bq).reshape(NE, P).T),
    }
    const_row = (bv @ Wo.T + bo).astype(np.float32)
    in_maps = [dict(base, xT=np.ascontiguousarray(xT[b])) for b in range(x.shape[0])]
    return in_maps, const_row


def kernel(x, Wq, bq, Wk, Wv, bv, Wo, bo):
    in_maps, const_row = prep_inputs(x, Wq, bq, Wk, Wv, bv, Wo, bo)
    nc = _get_nc(x.shape[1])
    res = bass_utils.run_bass_kernel_spmd(
        nc, in_maps, core_ids=list(range(len(in_maps))))
    out = np.stack([r["out"] for r in res.results])
    return (out + const_row[None, None, :]).astype(np.float32)


# revision 26
# speedup vs baseline: 1.3902x; 1.3902x over previous
"""Trainium2 Bass kernel for batched multi-head self-attention.

Problem: x [8, 1500, 768], 12 heads x 64 dims, torch-Linear style projections.
Strategy: data-parallel over batch (1 element per NeuronCore, 8 cores).

Per-core design (host pre-transposes everything; device does no transposes):
  - xT [768, 1500] bf16: projections contract over d on the partition axis.
  - All four weight matrices live in SBUF (loaded once, OUTSIDE the rep
    loop); Q^T / K^T for the current head-pair stay SBUF-resident too, so
    the steady-state body issues only 4 DMA instructions per rep
    (1 x-load + 3 output stores). DMA instructions carry a large fixed
    execution cost on this runtime, so everything else moves via
    engine-to-engine SBUF/PSUM traffic.
  - reps (for the delta-timing method) run as a HARDWARE loop
    (tc.For_i), so the program size - and hence the NEFF load cost per
    dispatch - is independent of the rep count: the reps-delta then
    measures pure per-rep device execution.
  - scores computed TRANSPOSED: scoresT[k, q] = K_h^T.T @ Q_h^T, two
    matmuls (one per head of the pair; contraction is dh=64).
  - exp on ScalarE straight out of PSUM ([128,1024] two-bank spans), no
    max subtraction (scores ~ N(0,1): fp32-safe).
  - softmax denominators ride as a 65th all-ones column of V inside the
    ctx matmul (ctxT psum = 64 ctx rows + 1 sums row).
  - normalization: reciprocal of the sums row, partition-broadcast via a
    1-contraction matmul against an all-ones stationary (PSUM out), then
    multiplied in during the ctx psum eviction. No DRAM roundtrips.
  - output projection consumes ctx_normT [e, s] directly; the bv/bo
    contribution is a constant row (softmax rows sum to 1) added on host.

Matmul operands are bf16 (full PE rate at any moving size, ~0.4% rel
error per matmul, well inside the 2e-2 gate); all PSUM accumulation is
fp32.
"""

import numpy as np
from contextlib import ExitStack

import ml_dtypes

import concourse.bass as bass
import concourse.bacc as bacc
import concourse.tile as tile
from concourse import mybir
from concourse import bass_utils

F32 = mybir.dt.float32
F32R = mybir.dt.float32r
BF16 = mybir.dt.bfloat16
AF = mybir.ActivationFunctionType
OP = mybir.AluOpType

P = 128
D = 768
H = 12
DH = 64
NE = D // P          # 6 e-chunks (head pairs)
ND = D // P          # 6 d-chunks
SCALE = 0.125
S_FULL = 1500
QB = 512
EH = 384             # half of D for the V projection moving dim


def _chunks(total, size):
    out = []
    o = 0
    while o < total:
        out.append((o, min(size, total - o)))
        o += size
    return out


def _qblocks(S):
    # Plain chunking (last block ragged). An overlapped-uniform-512
    # variant was measured SLOWER (1.61ms vs 1.28ms): the overlap
    # region's WAW ordering dependencies cost more than the ragged
    # paths' extra instructions save.
    return _chunks(S, QB)


def build_attention(tc, ctx, xT, wqT, wkT, wvT, woT, bqs, out, S, reps=1):
    """Emit the single-core attention program.

    xT:  [D, S] bf16 DRAM     (x^T for this batch element)
    wqT/wkT/wvT/woT: [D, D] bf16 DRAM  (W.T of the torch-Linear weights)
    bqs: [P, NE] f32 DRAM     (0.125*bq laid out [partition, e-chunk])
    out: [S, D] f32 DRAM      (missing the constant bv@Wo.T+bo row)
    """
    nc = tc.nc
    SC = _chunks(S, P)            # k-chunks, e.g. 11x128 + 92
    QBS = _qblocks(S)
    NSC = len(SC)

    const = ctx.enter_context(tc.tile_pool(name="const", bufs=1))
    qkv = ctx.enter_context(tc.tile_pool(name="qkv", bufs=1))
    gen_ps = ctx.enter_context(tc.tile_pool(name="gen_ps", bufs=2, space="PSUM"))
    sc_ps = ctx.enter_context(tc.tile_pool(name="sc_ps", bufs=2, space="PSUM"))
    ctx_ps = ctx.enter_context(tc.tile_pool(name="ctx_ps", bufs=1, space="PSUM"))
    e_pool = ctx.enter_context(tc.tile_pool(name="epool", bufs=2))
    sr_pool = ctx.enter_context(tc.tile_pool(name="srp", bufs=2))
    ctxn_pool = ctx.enter_context(tc.tile_pool(name="ctxn", bufs=3))
    rc_pool = ctx.enter_context(tc.tile_pool(name="rcp", bufs=3))
    out_sb_pool = ctx.enter_context(tc.tile_pool(name="outsb", bufs=3))
    kt_pool = ctx.enter_context(tc.tile_pool(name="ktp", bufs=2))
    x_pool = ctx.enter_context(tc.tile_pool(name="xp", bufs=1))

    # ---- persistent operands, loaded ONCE (outside the rep loop) ----
    bq_sb = const.tile([P, NE], F32)
    nc.sync.dma_start(out=bq_sb[:], in_=bqs)
    w_sbs = {}
    for key, wdram in (("q", wqT), ("k", wkT), ("v", wvT), ("o", woT)):
        w_sb = const.tile([P, ND, D], BF16, name=f"w_{key}")
        for dc in range(ND):
            nc.sync.dma_start(out=w_sb[:, dc, :],
                              in_=wdram[dc * P:(dc + 1) * P, :])
        w_sbs[key] = w_sb
    ones_sb = const.tile([1, DH], F32)
    nc.vector.memset(ones_sb[:], 1.0)

    V = qkv.tile([P, NSC, H * (DH + 1)], BF16)   # per-head 65th ones column
    # Fill all of V with 1.0 once: the projection evictions overwrite the
    # 64 data columns per head, leaving column DH as the all-ones column
    # that accumulates softmax denominators in the ctx matmul. bf16 memset
    # isn't a valid ISA op, so memset the f32-bitcast view with the bit
    # pattern of two packed bf16 1.0s (0x3F803F80).
    two_bf16_ones = float(np.frombuffer(
        np.uint32(0x3F803F80).tobytes(), dtype=np.float32)[0])
    nc.vector.memset(V[:, :, :].bitcast(F32), two_bf16_ones)

    # reps as a HARDWARE loop: program size (NEFF size / per-dispatch
    # load cost) is independent of the rep count, so the reps-delta
    # wall-clock measurement isolates pure per-rep device execution.
    def body():
        _emit_body(tc, nc, xT, out, S, SC, QBS, NSC,
                   kt_pool, V, bq_sb, w_sbs, ones_sb, gen_ps, sc_ps,
                   ctx_ps, e_pool, sr_pool, ctxn_pool, rc_pool,
                   out_sb_pool, x_pool)

    if reps == 1:
        body()
    else:
        with tc.For_i(0, reps, 1):
            body()


def _emit_body(tc, nc, xT, out, S, SC, QBS, NSC,
               kt_pool, V, bq_sb, w_sbs, ones_sb, gen_ps, sc_ps,
               ctx_ps, e_pool, sr_pool, ctxn_pool, rc_pool, out_sb_pool,
               x_pool):

    # x^T for this rep: 6 flat 2D DMAs (contiguous source rows lower to
    # single hardware descriptors; a fancy 3D pattern costs ~30x more).
    xT_sb = x_pool.tile([P, ND, S], BF16, tag="xt", name="xT_sb")
    for dc in range(ND):
        nc.sync.dma_start(out=xT_sb[:, dc, :],
                          in_=xT[dc * P:(dc + 1) * P, :])

    SPAN = 6              # k-chunks per exp batch (NSC=12 -> 2 batches)

    def scores_kc(kc, q0, qw, kt_t, qt_t, sraw):
        """QK^T for one k-chunk + PSUM->SBUF eviction (fp16 staging).
        Evictions alternate DVE / ScalarE so the ~1.6us blocked-semaphore
        wake latency is split between two engines instead of serializing
        on one, and the batched exp frees ScalarE of per-k-chunk waits."""
        (k0, kw) = SC[kc]
        sp = sc_ps.tile([P, 1024], F32, tag="sc", name="sp")
        for hi in range(2):
            nc.tensor.matmul(
                sp[:kw, hi * 512:hi * 512 + qw],
                kt_t[hi * DH:(hi + 1) * DH, k0:k0 + kw],
                qt_t[hi * DH:(hi + 1) * DH, q0:q0 + qw],
                start=True, stop=True)
        si = kc % SPAN
        if qw == 512:
            if kc % 2 == 0:
                nc.vector.tensor_copy(out=sraw[:kw, si, :], in_=sp[:kw, :])
            else:
                nc.scalar.copy(out=sraw[:kw, si, :], in_=sp[:kw, :])
        else:
            for hi in range(2):
                dst = sraw[:kw, si, hi * 512:hi * 512 + qw]
                src = sp[:kw, hi * 512:hi * 512 + qw]
                if kc % 2 == 0:
                    nc.vector.tensor_copy(out=dst, in_=src)
                else:
                    nc.scalar.copy(out=dst, in_=src)

    def exp_span(qw, sraw, sp0, nsc_span):
        """One giant exp over a whole staged span (up to 6 k-chunks) --
        tens of ACT instructions per rep instead of 216. Only staged
        (initialized) regions are read: full-row chunks go in one AP, a
        ragged-row tail chunk gets its own."""
        e_all = e_pool.tile([P, SPAN, 1024], BF16, tag="e", name="e_all")
        nf = sum(1 for si in range(nsc_span) if SC[sp0 + si][1] == P)
        col_ranges = ([(0, 1024)] if qw == 512
                      else [(0, qw), (512, qw)])
        for (c0, cw) in col_ranges:
            if nf:
                nc.scalar.activation(out=e_all[:, 0:nf, c0:c0 + cw],
                                     in_=sraw[:, 0:nf, c0:c0 + cw],
                                     func=AF.Exp)
            if nf < nsc_span:
                kw = SC[sp0 + nf][1]
                nc.scalar.activation(out=e_all[:kw, nf:nf + 1, c0:c0 + cw],
                                     in_=sraw[:kw, nf:nf + 1, c0:c0 + cw],
                                     func=AF.Exp)
        return e_all

    def ctx_kc(kc, qw, pr, cps, e_all):
        (k0, kw) = SC[kc]
        si = kc % SPAN
        for hi in range(2):
            h = 2 * pr + hi
            nc.tensor.matmul(
                cps[:, hi * 512:hi * 512 + qw],
                V[:kw, kc, h * (DH + 1):(h + 1) * (DH + 1)],
                e_all[:kw, si, hi * 512:hi * 512 + qw],
                start=(kc == 0), stop=(kc == NSC - 1))

    def phase2_end(qw, pr, cps, cn):
        for hi in range(2):
            # reciprocal of the denominators row, then partition-broadcast
            # it across the 64 ctx rows with a 1-contraction matmul. The
            # ctx rows evict PSUM->SBUF in parallel (a DVE op may read at
            # most one PSUM operand, so the multiply pairs SBUF x PSUM).
            co = hi * 512
            rc = rc_pool.tile([1, 512], F32R, tag="rc", name="rc")
            # f32r out is storage-identical to f32 here; the reciprocal
            # itself computes in fp32 internally.
            with nc.allow_low_precision(reason="f32r reciprocal, fp32 bits"):
                nc.vector.reciprocal(out=rc[:, 0:qw],
                                     in_=cps[DH:DH + 1, co:co + qw])
            craw = rc_pool.tile([DH, 512], F32, tag="craw", name="craw")
            nc.vector.tensor_copy(out=craw[:, 0:qw], in_=cps[0:DH, co:co + qw])
            rb_ps = gen_ps.tile([P, 512], F32, tag="mm", name="rb_ps")
            nc.tensor.matmul(rb_ps[:DH, 0:qw], ones_sb[:, :].bitcast(F32R),
                             rc[:, 0:qw], start=True, stop=True)
            nc.vector.tensor_tensor(
                out=cn[hi * DH:(hi + 1) * DH, pr, 0:qw],
                in0=craw[:, 0:qw], in1=rb_ps[:DH, 0:qw], op=OP.mult)

    def phase2_pair(q0, qw, pr, kt_t, qt_t, cn, interleave_v=False):
        """Per-(pair, q-block) attention, software-pipelined: all scores
        of a span are computed and staged before its exp; all ctx matmuls
        run after both exps are in flight, so every PE wait on ACT output
        is already satisfied when it issues."""
        cps = ctx_ps.tile([DH + 1, 1024], F32, tag="ctx", name="cps")
        e_spans = []
        for sp0 in range(0, NSC, SPAN):
            nsc_span = min(SPAN, NSC - sp0)
            sraw = sr_pool.tile([P, SPAN, 1024], mybir.dt.float16,
                                tag="sr", name="sraw")
            for kc in range(sp0, sp0 + nsc_span):
                if interleave_v:
                    emit_v_chunk(kc, *SC[kc])
                scores_kc(kc, q0, qw, kt_t, qt_t, sraw)
            e_spans.append(exp_span(qw, sraw, sp0, nsc_span))
        for kc in range(NSC):
            ctx_kc(kc, qw, pr, cps, e_spans[kc // SPAN])
        phase2_end(qw, pr, cps, cn)

    def phase3(q0, qw, cn):
        for (s0, sw) in _chunks(qw, P):
            ot = out_sb_pool.tile([P, D], F32, tag="ot", name="ot")
            for (o0, ow) in ((0, 512), (512, 256)):
                op_t = gen_ps.tile([P, 512], F32, tag="mm", name="op_t")
                for ec in range(NE):
                    nc.tensor.matmul(
                        op_t[:sw, :ow],
                        cn[:, ec, s0:s0 + sw],
                        w_sbs["o"][:, ec, o0:o0 + ow],
                        start=(ec == 0), stop=(ec == NE - 1))
                nc.vector.tensor_copy(out=ot[:sw, o0:o0 + ow],
                                      in_=op_t[:sw, :ow])
            # flat 2D store: contiguous destination rows -> cheap DMA
            nc.sync.dma_start(out=out[q0 + s0:q0 + s0 + sw, :],
                              in_=ot[:sw, :])

    def emit_kq(ec):
        kt_t = kt_pool.tile([P, S], BF16, tag="kt", name=f"kt{ec}")
        qt_t = kt_pool.tile([P, S], BF16, tag="qt", name=f"qt{ec}")
        for kind in ("q", "k"):
            w_sb = w_sbs[kind]
            for (q0, qw) in QBS:
                ps = gen_ps.tile([P, 512], F32, tag="mm", name="kq_ps")
                for dc in range(ND):
                    nc.tensor.matmul(
                        ps[:, :qw],
                        w_sb[:, dc, ec * P:(ec + 1) * P],
                        xT_sb[:, dc, q0:q0 + qw],
                        start=(dc == 0), stop=(dc == ND - 1))
                if kind == "q":
                    nc.vector.tensor_scalar(
                        out=qt_t[:, q0:q0 + qw], in0=ps[:, :qw],
                        scalar1=SCALE, scalar2=bq_sb[:, ec:ec + 1],
                        op0=OP.mult, op1=OP.add)
                else:
                    nc.vector.tensor_copy(out=kt_t[:, q0:q0 + qw],
                                          in_=ps[:, :qw])
        return kt_t, qt_t

    def emit_v_chunk(sc, s0, sw):
        for eh in range(D // EH):
            ps = gen_ps.tile([P, 512], F32, tag="mm", name="v_ps")
            for dc in range(ND):
                nc.tensor.matmul(
                    ps[:sw, :EH],
                    xT_sb[:, dc, s0:s0 + sw],
                    w_sbs["v"][:, dc, eh * EH:(eh + 1) * EH],
                    start=(dc == 0), stop=(dc == ND - 1))
            vh = V[:sw, sc, :].rearrange("p (h w) -> p h w", w=DH + 1)
            nc.vector.tensor_copy(
                out=vh[:, eh * (EH // DH):(eh + 1) * (EH // DH), 0:DH],
                in_=ps[:sw, :EH].rearrange("p (h w) -> p h w", w=DH))

    # pr-major emission (emission order IS program order under Tile):
    # each head-pair's K/Q projection is followed by that pair's
    # attention over ALL q-blocks. The V pass interleaves chunk-by-chunk
    # with the very first pair's scores. Each q-block's output
    # projection is emitted right after its last pair.
    cns = [ctxn_pool.tile([P, NE, 512], BF16, tag="cn", name=f"cn{_q}")
           for _q in range(len(QBS))]
    for pr in range(NE):
        kt_t, qt_t = emit_kq(pr)
        for qi, (q0, qw) in enumerate(QBS):
            phase2_pair(q0, qw, pr, kt_t, qt_t, cns[qi],
                        interleave_v=(pr == 0 and qi == 0))
            if pr == NE - 1:
                phase3(q0, qw, cns[qi])


def build_nc(S=S_FULL, reps=1):
    nc = bacc.Bacc("TRN2", target_bir_lowering=False, debug=False,
                   enable_asserts=False, num_devices=1)
    xT = nc.dram_tensor("xT", [D, S], BF16, kind="ExternalInput").ap()
    wqT = nc.dram_tensor("wqT", [D, D], BF16, kind="ExternalInput").ap()
    wkT = nc.dram_tensor("wkT", [D, D], BF16, kind="ExternalInput").ap()
    wvT = nc.dram_tensor("wvT", [D, D], BF16, kind="ExternalInput").ap()
    woT = nc.dram_tensor("woT", [D, D], BF16, kind="ExternalInput").ap()
    bqs = nc.dram_tensor("bqs", [P, NE], F32, kind="ExternalInput").ap()
    out = nc.dram_tensor("out", [S, D], F32, kind="ExternalOutput").ap()
    with tile.TileContext(nc) as tc:
        with ExitStack() as ctx:
            build_attention(tc, ctx, xT, wqT, wkT, wvT, woT, bqs, out, S, reps)
    nc.compile()
    return nc


_NC_CACHE = {}


def _get_nc(S=S_FULL, reps=1):
    if (S, reps) not in _NC_CACHE:
        _NC_CACHE[(S, reps)] = build_nc(S, reps)
    return _NC_CACHE[(S, reps)]


def prep_inputs(x, Wq, bq, Wk, Wv, bv, Wo, bo):
    x = np.asarray(x, dtype=np.float32)
    Wq = np.asarray(Wq, dtype=np.float32)
    Wk = np.asarray(Wk, dtype=np.float32)
    Wv = np.asarray(Wv, dtype=np.float32)
    Wo = np.asarray(Wo, dtype=np.float32)
    bq = np.asarray(bq, dtype=np.float32)
    bv = np.asarray(bv, dtype=np.float32)
    bo = np.asarray(bo, dtype=np.float32)
    bf = ml_dtypes.bfloat16
    xT = np.ascontiguousarray(x.transpose(0, 2, 1)).astype(bf)
    base = {
        "wqT": np.ascontiguousarray(Wq.T).astype(bf),
        "wkT": np.ascontiguousarray(Wk.T).astype(bf),
        "wvT": np.ascontiguousarray(Wv.T).astype(bf),
        "woT": np.ascontiguousarray(Wo.T).astype(bf),
        "bqs": np.ascontiguousarray((SCALE * bq).reshape(NE, P).T),
    }
    const_row = (bv @ Wo.T + bo).astype(np.float32)
    in_maps = [dict(base, xT=np.ascontiguousarray(xT[b])) for b in range(x.shape[0])]
    return in_maps, const_row


def kernel(x, Wq, bq, Wk, Wv, bv, Wo, bo):
    in_maps, const_row = prep_inputs(x, Wq, bq, Wk, Wv, bv, Wo, bo)
    nc = _get_nc(x.shape[1])
    res = bass_utils.run_bass_kernel_spmd(
        nc, in_maps, core_ids=list(range(len(in_maps))))
    out = np.stack([r["out"] for r in res.results])
    return (out + const_row[None, None, :]).astype(np.float32)
